# revision 4
# baseline (speedup 1.0000x reference)
"""DiffJPEG forward (16x3x512x512, quality=80) on 8 TRN2 NeuronCores.

Strategy: pure data-parallel over batch (2 images/core). Per core the JPEG
pipeline runs on-chip as 4 PE matmul stages (form-b / form-a alternation —
form-b stages use the data as the stationary operand, transposing for free),
everything in fp16 except the fp32 PSUM accumulators:

  S1 (form-b):  G1 = X^T A^T            vertical DCT (1 cyc/row at N=128)
  S2 (form-a):  F^T = sum_c' L[c,c'] G1_c'   horizontal DCT + fused 255*W_ycc
                                        color mix (N=1024)
  quant      :  t16 = F*(1/q) stored fp16; round via fp16 magic (+1536/-1536
                4x-mode DVE TSPs: the fp16 STORE does the rounding since
                |t|<=455 < 512 keeps t+1536 in the ulp=1 range); dequant is
                one fp16 2x-mode tensor_tensor by the q pattern
  S3 (form-b):  G3 = Q^T-chain = tq M per block
  S4 (form-a):  R = sum_c V[chan,c] M^T G3_c + 128/255 (fused inverse color
                                        mix, rank-1 DC-row bias)
  out        :  plain psum->sbuf fp16 copy; the [0,1] clip runs on HOST after
                the gather (bit-identical: clip(fp16(x)) == fp16-store of
                clip(x) for this range), removing 12 full-tile clip ops.

Precision: tolerance is 2e-2 L2-rel; the fp16-magic pipeline measures 8.8e-3
in exact numpy emulation. Level shifts / color biases collapse into
DC-coefficient corrections (dca pattern on the forward, a rank-1 DC-row bias
on G3_y for the inverse, exact because beta*m0 == 128/255 by construction);
quality-dependent quant tables arrive as a tiny per-core [128,103] input pack
(global slice index = 6*core + local_slice selects luma/chroma).

Scheduling: PSUM tiles are 2-bank [128,1024]; S2/S4 use single N=1024
matmuls per accumulation group. PSUM-reading work is split over the only two
engines with a PSUM port (DVE: quant TT1; ACT/DVE: S1/S3/S4 psum->sbuf
copies per QCONF); the SBUF-only fp16 quant ops run at DVE 4x/2x perf modes
or on GPSIMD per QCONF. Junk matmuls at t=0 hold the PE p-state ramp through
the input-load head; S3/S4 interleave per column-pair so output DMA streams
instead of flushing at the tail.
"""

import numpy as np

import concourse.bass as bass
import concourse.mybir as mybir
import concourse.tile as tile
from concourse import bacc
from concourse.bass_utils import run_bass_kernel_spmd

N_CORES = 8
BS = 16
IMGS_PER_CORE = BS // N_CORES          # 2
SLICES = IMGS_PER_CORE * 3             # 6
MAGIC16 = 1536.0                       # fp16 round-to-nearest at ulp=1

F32 = mybir.dt.float32
F16 = mybir.dt.float16
COPY = mybir.ActivationFunctionType.Copy
IDENT = mybir.ActivationFunctionType.Identity

_LUM = np.array([[16,11,10,16,24,40,51,61],[12,12,14,19,26,58,60,55],[14,13,16,24,40,57,69,56],[14,17,22,29,51,87,80,62],[18,22,37,56,68,109,103,77],[24,35,55,64,81,104,113,92],[49,64,78,87,103,121,120,101],[72,92,95,98,112,100,103,99]], np.float32)
_CHROM = np.array([[17,18,24,47,99,99,99,99],[18,21,26,66,99,99,99,99],[24,26,56,99,99,99,99,99],[47,66,99,99,99,99,99,99],[99,99,99,99,99,99,99,99],[99,99,99,99,99,99,99,99],[99,99,99,99,99,99,99,99],[99,99,99,99,99,99,99,99]], np.float32)
_WYCC = np.array([[0.299, 0.587, 0.114], [-0.1687, -0.3313, 0.5], [0.5, -0.4187, -0.0813]], np.float32)
# inverse color terms: out_chan <- sum of coef * rec_channel (y=0, cb=1, cr=2)
_S4TERMS = [
    [(0, 1.0), (2, 1.402)],                       # r
    [(0, 1.0), (1, -0.34414), (2, -0.71414)],     # g
    [(0, 1.0), (1, 1.772)],                       # b
]

# Engine assignment (A=ACT, P=GPSIMD, V=DVE), indexed per tile (cycled).
# s1cp/s3cp/s4cp: psum->sbuf copies (12 each, ti = 6*im + 2*c|chan-ish + sp).
# m1/m2: fp16 magic add/sub at channel granularity (6, tc = 3*im + c).
# deq: fp16 dequant TT at channel granularity (6).
# dca: Y-channel DC adjust (4, 2*im + sp).
QCONF = {"sched": "fwd2",
         "s1cp": ["A"],
         "s3cp": ["A"],
         "s4cp": ["V", "A", "A", "V", "A", "A"],
         "m1":   ["P", "V", "P", "V", "P", "V"],
         "m2":   ["V"],
         "deq":  ["V"],
         "dca":  ["P"],
         "warm": 12, "s4ord": (1, 0, 2),
         "s3ord": (0, 1, 2)}


def _dct_mat():
    k = np.arange(8)[:, None]
    n = np.arange(8)[None, :]
    norm = np.where(k == 0, np.sqrt(1.0 / 8.0), np.sqrt(2.0 / 8.0))
    return (norm * np.cos(np.pi / 8.0 * (n + 0.5) * k)).astype(np.float32)


def _qtables(quality):
    q = max(1, min(100, int(quality)))
    scale = 5000.0 / q if q < 50 else 200.0 - 2.0 * q
    tbs = np.stack([_LUM, _CHROM]) * np.float32(scale)
    return np.clip((tbs + 50.0) / 100.0, 1.0, 255.0).astype(np.float32)


def _host_constants():
    M = _dct_mat()
    BD = np.kron(np.eye(16, dtype=np.float32), M)       # kron(I16, M)
    BDT = np.ascontiguousarray(BD.T)                    # kron(I16, M^T)

    s13w = np.concatenate([BDT, BD], axis=1).astype(np.float16)  # [128,256]

    s2w = np.zeros((128, 9 * 128), np.float16)          # [p, 9n]: BDT*255*W
    for c in range(3):
        for cp in range(3):
            s2w[:, 128 * (3 * c + cp) : 128 * (3 * c + cp) + 128] = (
                BDT * np.float32(255.0 * _WYCC[c, cp])).astype(np.float16)

    s4w = np.zeros((128, 7 * 128), np.float16)          # [p, 7n]: BD*coef/255
    s4idx = {}
    wi = 0
    for chan in range(3):
        for (csrc, coef) in _S4TERMS[chan]:
            s4idx[(chan, csrc)] = wi
            s4w[:, 128 * wi : 128 * wi + 128] = (
                BD * np.float32(coef / 255.0)).astype(np.float16)
            wi += 1

    m128 = np.arange(128)
    # +128/255 output bias, folded into the y-channel S3-out copy: adding
    # beta at DCT-row-0 partitions of G3_y contributes beta*m0 per pixel
    # through every channel's (chan,0) S4 weight, where m0 is that weight's
    # DC-row entry. Choose beta so beta*m0 == 128/255 exactly.
    m0 = float(np.float32(s4w[0, 128 * s4idx[(0, 0)]]))
    beta = np.float32(np.float64(128.0 / 255.0) / m0)
    s3b = (np.float32(beta) * (m128 % 8 == 0)).astype(np.float32)[:, None]  # [128,1]
    return dict(s13w=s13w, s2w=s2w, s4w=s4w, s4idx=s4idx, s3b=s3b)


def _quant_inputs(quality, core, s3b):
    """Per-core quant-pattern pack [128, 103]: columns are rq [6x8], qq [6x8],
    dca [6], s3b [1].

    Quant runs on F^T laid out [v (partition), u (free)]:
    pattern value at (p, j) = qt[u=j, v=p%8]."""
    qt = _qtables(quality)
    pack = np.zeros((128, 2 * SLICES * 8 + SLICES + 1), np.float32)
    p = np.arange(128)
    for i in range(SLICES):
        g = 6 * core + i                      # global flattened (b,c) slice
        tab = qt[0] if g < BS else qt[1]
        # [128,8]: [p, j] = tab[j, p%8]
        pack[:, 8 * i : 8 * i + 8] = (1.0 / tab.astype(np.float64))[:, p % 8].T.astype(np.float32)
        pack[:, 8 * (SLICES + i) : 8 * (SLICES + i) + 8] = tab[:, p % 8].T
        # -1024 * (1/q[0,0]): the Y-channel DC level-shift applied post-rq-mult,
        # nonzero only on v%8==0 partitions (add of 0 elsewhere is a no-op)
        pack[p % 8 == 0, 16 * SLICES + i] = np.float32(
            -1024.0 * float(pack[0, 8 * i]))
    pack[:, 17 * SLICES] = s3b[:, 0]
    return pack


def _pick(lst, i):
    return lst[i % len(lst)]


def _trace():
    hc = _host_constants()
    nc = bacc.Bacc("TRN2", target_bir_lowering=False, debug=False)

    NQ = 2 * SLICES * 8 + SLICES + 1
    img_d = nc.dram_tensor("img", [SLICES, 512, 512], F16, kind="ExternalInput").ap()
    qp_d = nc.dram_tensor("qpack", [128, NQ], F32, kind="ExternalInput").ap()
    s13w_d = nc.dram_tensor("s13w", [128, 256], F16, kind="ExternalInput").ap()
    s2w_d = nc.dram_tensor("s2w", [128, 9 * 128], F16, kind="ExternalInput").ap()
    s4w_d = nc.dram_tensor("s4w", [128, 7 * 128], F16, kind="ExternalInput").ap()
    # fp16 wire format for the output (host upcasts to f32 and clips).
    # Layout [im*2+sp, 128, (chan, slab, col)] matches the staging tiles so
    # each channel ships as one contiguous [128,1024] DMA.
    out_d = nc.dram_tensor("out", [2 * IMGS_PER_CORE, 128, 3072], F16, kind="ExternalOutput").ap()

    s4idx = hc["s4idx"]

    with tile.TileContext(nc) as tc:
        with (
            tc.tile_pool(name="wts", bufs=1) as wp,
            tc.tile_pool(name="img", bufs=2) as imp,
            tc.tile_pool(name="g1", bufs=2) as g1p,
            tc.tile_pool(name="tq", bufs=2) as tqp,
            tc.tile_pool(name="dq", bufs=2) as dqp,
            tc.tile_pool(name="g3", bufs=2) as g3p,
            tc.tile_pool(name="ost", bufs=3) as op,
            tc.tile_pool(name="psA", bufs=2, space="PSUM") as psAp,
            tc.tile_pool(name="psB", bufs=2, space="PSUM") as psBp,
        ):
            # img tile free layout: (half mtp, slab s, col c) so each plane
            # arrives as two 256-col DMAs and S1 can start on the first half.
            def load_plane(t, sl):
                for mtp in range(2):
                    nc.sync.dma_start(
                        t[:, 1024 * mtp : 1024 * mtp + 1024]
                        .rearrange("p (s c) -> p s c", s=4),
                        img_d[sl, :, 256 * mtp : 256 * mtp + 256]
                        .rearrange("(s p) c -> p s c", p=128),
                    )

            def xslice(t, w, mt):
                """[128,128] stationary slice: slab w, cols 128mt..128mt+128."""
                base = 1024 * (mt // 2) + 256 * w + 128 * (mt % 2)
                return t[:, base : base + 128]

            s13w = wp.tile([128, 256], F16, tag="s13w")
            nc.sync.dma_start(s13w[:], s13w_d)
            s1w = s13w[:, 0:128]
            s3w = s13w[:, 128:256]

            # PE warmup: junk matmuls on a memset tile (no DMA dependency)
            # keep the tensor engine continuously busy through the preamble
            # and input-load head, so the p-state ramp completes before the
            # real pipeline starts.
            wgarb = wp.tile([128, 256], F16, tag="wgarb")
            nc.vector.memset(wgarb[:], 0.0)

            def s1fill(n):
                """Filler matmuls: absorb input-DMA pacing gaps and hold the
                PE ramp. Fresh pool tiles so psA rotation is not pinned."""
                done = 0
                while done < n:
                    k = min(4, n - done)
                    wu = psAp.tile([128, 1024], F32, tag="psA")
                    for r in range(k):
                        nc.tensor.matmul(
                            wu[:, 256 * r : 256 * r + 256],
                            wgarb[:, 0:128], wgarb[:],
                            start=True, stop=True,
                        )
                    done += k

            s1fill(QCONF.get("warm", 12))

            early_imgs = []
            for _c in range(3):
                _t = imp.tile([128, 2048], F16, tag=f"x{_c}")
                load_plane(_t, _c)
                early_imgs.append(_t)
            qpk = wp.tile([128, NQ], F32, tag="qpk")
            nc.sync.dma_start(qpk[:], qp_d)
            O_RQ, O_QQ, O_DCA, O_S3B = 0, SLICES * 8, 16 * SLICES, 17 * SLICES
            s2w = wp.tile([128, 9 * 128], F16, tag="s2w")
            # split upload: channel-0's three mix blocks land first so S2 can
            # start as soon as image 0 is resident
            nc.sync.dma_start(s2w[:, 0 : 3 * 128], s2w_d[:, 0 : 3 * 128])
            nc.sync.dma_start(s2w[:, 3 * 128 :], s2w_d[:, 3 * 128 :])
            s4w = wp.tile([128, 7 * 128], F16, tag="s4w")
            nc.sync.dma_start(s4w[:], s4w_d)
            # fp16 copy of the dequant patterns for the all-fp16 dequant mult
            qq16 = wp.tile([128, SLICES * 8], F16, tag="qq16")
            nc.scalar.activation(qq16[:], qpk[:, O_QQ : O_QQ + SLICES * 8], COPY)

            state = {}

            def _eng(code):
                return {"A": nc.scalar, "P": nc.gpsimd, "V": nc.vector}[code]

            def s_load(im):
                if im == 0:
                    state[("x", 0)] = early_imgs
                    return
                xt = []
                for c in range(3):
                    t = imp.tile([128, 2048], F16, tag=f"x{c}")
                    load_plane(t, 3 * im + c)
                    xt.append(t)
                state[("x", im)] = xt

            def s1(im, chans=(0, 1, 2)):
                """Vertical DCT: per (c, mt-pair) one [128,1024] psum of 8
                fp16 matmuls, then one psum->sbuf fp16 copy."""
                xt = state[("x", im)]
                g1 = state.setdefault(("g1", im), [None, None, None])
                for c in chans:
                    g_t = g1p.tile([128, 2048], F16, tag=f"g1_{c}")
                    g1[c] = g_t
                    for mtp in range(2):
                        ps = psAp.tile([128, 1024], F32, tag="psA")
                        for k in range(2):
                            mt = 2 * mtp + k
                            for w in range(4):
                                nc.tensor.matmul(
                                    ps[:, 512 * k + 128 * w : 512 * k + 128 * w + 128],
                                    xslice(xt[c], w, mt),
                                    s1w,
                                    start=True, stop=True,
                                )
                        dst = g_t[:, 1024 * mtp : 1024 * mtp + 1024]
                        eng = _pick(QCONF["s1cp"], 6 * im + 2 * c + mtp)
                        if eng == "V":
                            nc.vector.tensor_scalar_add(dst, ps[:], 0.0)
                        else:
                            nc.scalar.activation(dst, ps[:], COPY)
                    s1fill(QCONF.get("s1fill", 0))

            def s2q(im, chans=(0, 1, 2)):
                """Horizontal DCT + color mix + quantization.

                Per (c, sp): [128,1024] psum from 3 fp16 N=1024 matmuls, then
                TT1 (psum * 1/q -> fp16).  Magic round (two 4x fp16 TSPs) and
                the fp16 dequant TT run at channel granularity [128,2048]."""
                g1 = state[("g1", im)]
                dqt = state.setdefault(("q", im), [None, None, None])
                for c in chans:
                    tq = tqp.tile([128, 2048], F16, tag=f"tq_{c}")
                    dq = dqp.tile([128, 2048], F16, tag=f"dq_{c}")
                    dqt[c] = dq
                    sl = 3 * im + c
                    rqv = qpk[:, O_RQ + 8 * sl : O_RQ + 8 * sl + 8].rearrange("p (o j) -> p o j", o=1).broadcast_to((128, 256, 8))
                    qqv16 = qq16[:, 8 * sl : 8 * sl + 8].rearrange("p (o j) -> p o j", o=1).broadcast_to((128, 256, 8))
                    for sp_ in range(2):
                        ps = psBp.tile([128, 1024], F32, tag="psB")
                        for k in range(2):
                            s = 2 * sp_ + k
                            for cp in range(3):
                                nc.tensor.matmul(
                                    ps[:, 512 * k : 512 * k + 512],
                                    s2w[:, 128 * (3 * c + cp) : 128 * (3 * c + cp) + 128],
                                    g1[cp][:, 512 * s : 512 * s + 512],
                                    start=(cp == 0), stop=(cp == 2),
                                )
                        ts = tq[:, 1024 * sp_ : 1024 * sp_ + 1024]
                        # TT1: psum * (1/q) -> fp16 (the only psum-port op)
                        nc.vector.tensor_tensor(
                            ts.rearrange("p (a j) -> p a j", j=8),
                            ps[:].rearrange("p (a j) -> p a j", j=8),
                            rqv[:, 0:128, :], op=mybir.AluOpType.mult,
                        )
                        if c == 0:
                            deng = _pick(QCONF["dca"], 2 * im + sp_)
                            dcacol = qpk[:, O_DCA + sl : O_DCA + sl + 1]
                            if deng == "A":
                                nc.scalar.activation(
                                    ts[:, 0:1024:8], ts[:, 0:1024:8], IDENT,
                                    bias=dcacol)
                            else:
                                _eng(deng).tensor_scalar_add(
                                    ts[:, 0:1024:8], ts[:, 0:1024:8], dcacol)
                    tc_ = 3 * im + c
                    # fp16 magic round: the fp16 stores perform the rounding
                    m1 = _pick(QCONF["m1"], tc_)
                    if m1 == "A":
                        nc.scalar.activation(tq[:], tq[:], COPY, bias=MAGIC16)
                    else:
                        _eng(m1).tensor_scalar_add(tq[:], tq[:], MAGIC16)
                    m2 = _pick(QCONF["m2"], tc_)
                    if m2 == "A":
                        nc.scalar.activation(tq[:], tq[:], COPY, bias=-MAGIC16)
                    else:
                        _eng(m2).tensor_scalar_sub(tq[:], tq[:], MAGIC16)
                    # dequant: all-fp16 2x tensor_tensor by the q pattern
                    _eng(_pick(QCONF["deq"], tc_)).tensor_tensor(
                        dq[:].rearrange("p (a j) -> p a j", j=8),
                        tq[:].rearrange("p (a j) -> p a j", j=8),
                        qqv16, op=mybir.AluOpType.mult,
                    )

            def s3(im, mtps=(0, 1), chans=(0, 1, 2)):
                """Horizontal inverse DCT (fp16 form-b). Needs all of dq[c]."""
                dqt = state[("q", im)]
                g3 = state.setdefault(("g3", im), [None, None, None])
                for mtp in mtps:
                    for c in (QCONF.get("s3ord", chans) if len(chans) == 3 else chans):
                        if mtp == 0:
                            g3_t = g3p.tile([128, 2048], F16, tag=f"g3_{c}")
                            g3[c] = g3_t
                        ps = psAp.tile([128, 1024], F32, tag="psA")
                        for k in range(2):
                            mt = 2 * mtp + k
                            for c2 in range(4):
                                nc.tensor.matmul(
                                    ps[:, 512 * k + 128 * c2 : 512 * k + 128 * c2 + 128],
                                    dqt[c][:, 512 * c2 + 128 * mt : 512 * c2 + 128 * mt + 128],
                                    s3w,
                                    start=True, stop=True,
                                )
                        g3s = g3[c][:, 1024 * mtp : 1024 * mtp + 1024]
                        eng = _pick(QCONF["s3cp"], 6 * im + 2 * c + mtp)
                        if eng == "V":
                            if c == 0:
                                nc.vector.tensor_scalar_add(
                                    g3s, ps[:], qpk[:, O_S3B : O_S3B + 1])
                            else:
                                nc.vector.tensor_scalar_add(g3s, ps[:], 0.0)
                        elif c == 0:
                            nc.scalar.activation(
                                g3s, ps[:], IDENT,
                                bias=qpk[:, O_S3B : O_S3B + 1],
                            )
                        else:
                            nc.scalar.activation(g3s, ps[:], COPY)

            def s4(im, sps=(0, 1)):
                """Vertical inverse DCT + inverse color mix + store.

                Slab-pair sp only needs g3[:, 1024sp:+1024] = s3(im, mtp=sp).
                One plain psum->sbuf fp16 copy (clip runs on host), one
                [128,1024] DMA per channel."""
                g3 = state[("g3", im)]
                for sp_ in sps:
                    ot_t = op.tile([128, 3072], F16, tag="ot")
                    for chan in QCONF.get("s4ord", (0, 1, 2)):
                        terms = _S4TERMS[chan]
                        ps = psBp.tile([128, 1024], F32, tag="psB")
                        for k in range(2):
                            s = 2 * sp_ + k
                            for ti, (csrc, _) in enumerate(terms):
                                wi = s4idx[(chan, csrc)]
                                nc.tensor.matmul(
                                    ps[:, 512 * k : 512 * k + 512],
                                    s4w[:, 128 * wi : 128 * wi + 128],
                                    g3[csrc][:, 512 * s : 512 * s + 512],
                                    start=(ti == 0), stop=(ti == len(terms) - 1),
                                )
                        ots = ot_t[:, 1024 * chan : 1024 * chan + 1024]
                        ci = 6 * im + 3 * sp_ + chan
                        if _pick(QCONF["s4cp"], ci) == "A":
                            nc.scalar.activation(ots, ps[:], COPY)
                        else:
                            nc.vector.tensor_scalar_add(ots, ps[:], 0.0)
                        oqs = QCONF.get("oq", ["S"])
                        oq = _pick(oqs, ci)
                        oeng = {"S": nc.sync, "A": nc.scalar, "P": nc.gpsimd}[oq]
                        oeng.dma_start(
                            out_d[2 * im + sp_, :, 1024 * chan : 1024 * chan + 1024],
                            ots,
                        )

            # software-pipelined schedule; S3/S4 interleave per column-pair
            # so outputs stream early instead of flushing at the tail.
            s_load(0)
            s_load(1)
            if QCONF["sched"] == "fwd2":
                # both images' forward stages first (wide window for the
                # quant chains to hide behind PE work), then the inverses
                s1(0)
                s2q(0)
                s1(1)
                s2q(1)
                for im in (0, 1):
                    s3(im, mtps=(0,))
                    s4(im, sps=(0,))
                    s3(im, mtps=(1,))
                    s4(im, sps=(1,))
            elif QCONF["sched"] == "hyb":
                # image-1 forward partially interleaved so image-0 outputs
                # start early and output DMA spreads across the whole run
                s1(0)
                s2q(0)
                s1(1)
                s2q(1, chans=(0,))
                s3(0, mtps=(0,))
                s4(0, sps=(0,))
                s2q(1, chans=(1,))
                s3(0, mtps=(1,))
                s4(0, sps=(1,))
                s2q(1, chans=(2,))
                s3(1, mtps=(0,))
                s4(1, sps=(0,))
                s3(1, mtps=(1,))
                s4(1, sps=(1,))
            else:
                s1(0)
                s2q(0)
                s1(1)
                s3(0, mtps=(0,))
                s4(0, sps=(0,))
                s2q(1, chans=(0, 1))
                s3(0, mtps=(1,))
                s4(0, sps=(1,))
                s2q(1, chans=(2,))
                s3(1, mtps=(0,))
                s4(1, sps=(0,))
                s3(1, mtps=(1,))
                s4(1, sps=(1,))
    nc.compile()
    return nc, hc


_COMPILED = None


def _get_compiled():
    global _COMPILED
    if _COMPILED is None:
        _COMPILED = _trace()
    return _COMPILED


def kernel(img, quality):
    img = np.asarray(img)
    quality = int(np.asarray(quality))
    nc, hc = _get_compiled()

    img16 = np.ascontiguousarray(img.astype(np.float16))
    in_maps = []
    for core in range(N_CORES):
        qpack = _quant_inputs(quality, core, hc["s3b"])
        shard = np.ascontiguousarray(
            img16[IMGS_PER_CORE * core : IMGS_PER_CORE * (core + 1)].reshape(SLICES, 512, 512)
        )
        in_maps.append({
            "img": shard, "qpack": qpack,
            "s13w": hc["s13w"], "s2w": hc["s2w"], "s4w": hc["s4w"],
        })

    res = run_bass_kernel_spmd(nc, in_maps, core_ids=list(range(N_CORES)))
    # wire layout per core: [im*2+sp, p, (chan, slab, col)] -> [2,3,512,512]
    out = np.stack([res.results[c]["out"] for c in range(N_CORES)])
    out = out.reshape(N_CORES, IMGS_PER_CORE, 2, 128, 3, 2, 512)
    out = out.transpose(0, 1, 4, 2, 5, 3, 6)   # [core, im, ch, sp, s, p, col]
    out = np.ascontiguousarray(out).reshape(BS, 3, 512, 512).astype(np.float32)
    return np.clip(out, 0.0, 1.0)


if __name__ == "__main__":
    rng = np.random.default_rng(0)
    x = rng.random((BS, 3, 512, 512), dtype=np.float32)
    y = kernel(x, 80)
    print("kernel ran:", y.shape, y.dtype, float(y.min()), float(y.max()))


# revision 46
# speedup vs baseline: 1.0614x; 1.0614x over previous
"""DiffJPEG forward (16x3x512x512, quality=80) on 8 TRN2 NeuronCores.

Strategy: pure data-parallel over batch (2 images/core). Per core the JPEG
pipeline runs on-chip as 4 PE matmul stages (form-b / form-a alternation —
form-b stages use the data as the stationary operand, transposing for free),
everything in fp16 except the fp32 PSUM accumulators:

  S1 (form-b):  G1 = X^T A^T            vertical DCT (1 cyc/row at N=128)
  S2 (form-a):  F^T = sum_c' L[c,c'] G1_c'   horizontal DCT + fused 255*W_ycc
                                        color mix (N=1024)
  quant      :  t16 = F*(1/q) stored fp16; round via fp16 magic (+1536/-1536
                4x-mode DVE TSPs: the fp16 STORE does the rounding since
                |t|<=455 < 512 keeps t+1536 in the ulp=1 range); dequant is
                one fp16 2x-mode tensor_tensor by the q pattern
  S3 (form-b):  G3 = Q^T-chain = tq M per block
  S4 (form-a):  R = sum_c V[chan,c] M^T G3_c + 128/255 (fused inverse color
                                        mix, rank-1 DC-row bias)
  out        :  plain psum->sbuf fp16 copy; the [0,1] clip runs on HOST after
                the gather (bit-identical: clip(fp16(x)) == fp16-store of
                clip(x) for this range), removing 12 full-tile clip ops.

Precision: tolerance is 2e-2 L2-rel; the fp16-magic pipeline measures 8.8e-3
in exact numpy emulation. Level shifts / color biases collapse into
DC-coefficient corrections (dca pattern on the forward, a rank-1 DC-row bias
on G3_y for the inverse, exact because beta*m0 == 128/255 by construction);
quality-dependent quant tables arrive as a tiny per-core [128,103] input pack
(global slice index = 6*core + local_slice selects luma/chroma).

Scheduling: PSUM tiles are 2-bank [128,1024]; S2/S4 use single N=1024
matmuls per accumulation group. PSUM-reading work is split over the only two
engines with a PSUM port (DVE: quant TT1; ACT/DVE: S1/S3/S4 psum->sbuf
copies per QCONF); the SBUF-only fp16 quant ops run at DVE 4x/2x perf modes
or on GPSIMD per QCONF. Junk matmuls at t=0 hold the PE p-state ramp through
the input-load head; S3/S4 interleave per column-pair so output DMA streams
instead of flushing at the tail.
"""

import numpy as np

import concourse.bass as bass
import concourse.mybir as mybir
import concourse.tile as tile
from concourse import bacc
from concourse.bass_utils import run_bass_kernel_spmd

N_CORES = 8
BS = 16
IMGS_PER_CORE = BS // N_CORES          # 2
SLICES = IMGS_PER_CORE * 3             # 6
MAGIC16 = 1536.0                       # fp16 round-to-nearest at ulp=1

F32 = mybir.dt.float32
F16 = mybir.dt.float16
COPY = mybir.ActivationFunctionType.Copy
IDENT = mybir.ActivationFunctionType.Identity

_LUM = np.array([[16,11,10,16,24,40,51,61],[12,12,14,19,26,58,60,55],[14,13,16,24,40,57,69,56],[14,17,22,29,51,87,80,62],[18,22,37,56,68,109,103,77],[24,35,55,64,81,104,113,92],[49,64,78,87,103,121,120,101],[72,92,95,98,112,100,103,99]], np.float32)
_CHROM = np.array([[17,18,24,47,99,99,99,99],[18,21,26,66,99,99,99,99],[24,26,56,99,99,99,99,99],[47,66,99,99,99,99,99,99],[99,99,99,99,99,99,99,99],[99,99,99,99,99,99,99,99],[99,99,99,99,99,99,99,99],[99,99,99,99,99,99,99,99]], np.float32)
_WYCC = np.array([[0.299, 0.587, 0.114], [-0.1687, -0.3313, 0.5], [0.5, -0.4187, -0.0813]], np.float32)
# inverse color terms: out_chan <- sum of coef * rec_channel (y=0, cb=1, cr=2)
_S4TERMS = [
    [(0, 1.0), (2, 1.402)],                       # r
    [(0, 1.0), (1, -0.34414), (2, -0.71414)],     # g
    [(0, 1.0), (1, 1.772)],                       # b
]

# Engine assignment (A=ACT, P=GPSIMD, V=DVE), indexed per tile (cycled).
# s1cp/s3cp/s4cp: psum->sbuf copies (12 each, ti = 6*im + 2*c|chan-ish + sp).
# m1/m2: fp16 magic add/sub at channel granularity (6, tc = 3*im + c).
# deq: fp16 dequant TT at channel granularity (6).
# dca: Y-channel DC adjust (4, 2*im + sp).
QCONF = {"sched": "fwd2i", "s1cp": ["A"], "s3cp": ["A"], "s4cp": ["V", "V", "A"],
         "m1": ["V"], "m2": ["A", "V"], "deq": ["V", "P"], "dca": ["P", "A"],
         "warm": 8, "s4ord": (2, 0, 1), "s3ord": (0, 1, 2), "qgran": "sp",
         "oq": ["S", "P"], "s1fill": 0, "isplit": 1, "tailfine": None,
         "yreuse": 0, "hmajor": 0, "obatch": ["ch"]}


def _dct_mat():
    k = np.arange(8)[:, None]
    n = np.arange(8)[None, :]
    norm = np.where(k == 0, np.sqrt(1.0 / 8.0), np.sqrt(2.0 / 8.0))
    return (norm * np.cos(np.pi / 8.0 * (n + 0.5) * k)).astype(np.float32)


def _qtables(quality):
    q = max(1, min(100, int(quality)))
    scale = 5000.0 / q if q < 50 else 200.0 - 2.0 * q
    tbs = np.stack([_LUM, _CHROM]) * np.float32(scale)
    return np.clip((tbs + 50.0) / 100.0, 1.0, 255.0).astype(np.float32)


def _host_constants():
    """Weight constants; s13w/s2w/s4w also concatenated into one wpack."""
    M = _dct_mat()
    BD = np.kron(np.eye(16, dtype=np.float32), M)       # kron(I16, M)
    BDT = np.ascontiguousarray(BD.T)                    # kron(I16, M^T)

    s13w = np.concatenate([BDT, BD], axis=1).astype(np.float16)  # [128,256]

    s2w = np.zeros((128, 9 * 128), np.float16)          # [p, 9n]: BDT*255*W
    for c in range(3):
        for cp in range(3):
            s2w[:, 128 * (3 * c + cp) : 128 * (3 * c + cp) + 128] = (
                BDT * np.float32(255.0 * _WYCC[c, cp])).astype(np.float16)

    s4w = np.zeros((128, 7 * 128), np.float16)          # [p, 7n]: BD*coef/255
    s4idx = {}
    wi = 0
    for chan in range(3):
        for (csrc, coef) in _S4TERMS[chan]:
            s4idx[(chan, csrc)] = wi
            s4w[:, 128 * wi : 128 * wi + 128] = (
                BD * np.float32(coef / 255.0)).astype(np.float16)
            wi += 1

    m128 = np.arange(128)
    # +128/255 output bias, folded into the y-channel S3-out copy: adding
    # beta at DCT-row-0 partitions of G3_y contributes beta*m0 per pixel
    # through every channel's (chan,0) S4 weight, where m0 is that weight's
    # DC-row entry. Choose beta so beta*m0 == 128/255 exactly.
    m0 = float(np.float32(s4w[0, 128 * s4idx[(0, 0)]]))
    beta = np.float32(np.float64(128.0 / 255.0) / m0)
    s3b = (np.float32(beta) * (m128 % 8 == 0)).astype(np.float32)[:, None]  # [128,1]

    # Y-reuse S2 weights: Cb = a*(B - Y), Cr = b*(R - Y) with a = .5/.886,
    # b = .5/.701 (exact JPEG chroma identities): one data stream + one
    # F_Y-passthrough (scaled identity) instead of three mix streams.
    a = np.float64(_WYCC[1, 2]) / np.float64(1.0 - _WYCC[0, 2])
    b_ = np.float64(_WYCC[2, 0]) / np.float64(1.0 - _WYCC[0, 0])
    I128 = np.eye(128, dtype=np.float32)
    s2yr = np.concatenate([
        BDT * np.float32(255.0 * a), I128 * np.float32(-a),
        BDT * np.float32(255.0 * b_), I128 * np.float32(-b_),
    ], axis=1).astype(np.float16)                        # [128, 4*128]

    wpack = np.ascontiguousarray(np.concatenate([s13w, s2w, s4w, s2yr], axis=1))
    return dict(wpack=wpack, s4idx=s4idx, s3b=s3b)


def _quant_inputs(quality, core, s3b):
    """Per-core quant-pattern pack [128, 103]: columns are rq [6x8], qq [6x8],
    dca [6], s3b [1].

    Quant runs on F^T laid out [v (partition), u (free)]:
    pattern value at (p, j) = qt[u=j, v=p%8]."""
    qt = _qtables(quality)
    pack = np.zeros((128, 2 * SLICES * 8 + SLICES + 1), np.float32)
    p = np.arange(128)
    for i in range(SLICES):
        g = 6 * core + i                      # global flattened (b,c) slice
        tab = qt[0] if g < BS else qt[1]
        # [128,8]: [p, j] = tab[j, p%8]
        pack[:, 8 * i : 8 * i + 8] = (1.0 / tab.astype(np.float64))[:, p % 8].T.astype(np.float32)
        pack[:, 8 * (SLICES + i) : 8 * (SLICES + i) + 8] = tab[:, p % 8].T
        # -1024 * (1/q[0,0]): the Y-channel DC level-shift applied post-rq-mult,
        # nonzero only on v%8==0 partitions (add of 0 elsewhere is a no-op)
        pack[p % 8 == 0, 16 * SLICES + i] = np.float32(
            -1024.0 * float(pack[0, 8 * i]))
    pack[:, 17 * SLICES] = s3b[:, 0]
    return pack


def _pick(lst, i):
    return lst[i % len(lst)]


def _trace():
    hc = _host_constants()
    nc = bacc.Bacc("TRN2", target_bir_lowering=False, debug=False)

    NQ = 2 * SLICES * 8 + SLICES + 1
    NW = 256 + 9 * 128 + 7 * 128 + 4 * 128  # s13w | s2w | s4w | s2yr
    img_d = nc.dram_tensor("img", [SLICES, 512, 512], F16, kind="ExternalInput").ap()
    qp_d = nc.dram_tensor("qpack", [128, NQ], F32, kind="ExternalInput").ap()
    wpack_d = nc.dram_tensor("wpack", [128, NW], F16, kind="ExternalInput").ap()
    # fp16 wire format for the output (host upcasts to f32 and clips).
    # Layout [im*2+sp, 128, (chan, slab, col)] matches the staging tiles so
    # each channel ships as one contiguous [128,1024] DMA.
    out_d = nc.dram_tensor("out", [2 * IMGS_PER_CORE, 128, 3072], F16, kind="ExternalOutput").ap()

    s4idx = hc["s4idx"]

    with tile.TileContext(nc) as tc:
        with (
            tc.tile_pool(name="wts", bufs=1) as wp,
            tc.tile_pool(name="img", bufs=2) as imp,
            tc.tile_pool(name="g1", bufs=2) as g1p,
            tc.tile_pool(name="fy", bufs=2) as fyp,
            tc.tile_pool(name="tq", bufs=2) as tqp,
            tc.tile_pool(name="dq", bufs=2) as dqp,
            tc.tile_pool(name="g3", bufs=2) as g3p,
            tc.tile_pool(name="ost", bufs=3) as op,
            tc.tile_pool(name="psA", bufs=2, space="PSUM") as psAp,
            tc.tile_pool(name="psB", bufs=2, space="PSUM") as psBp,
        ):
            # img tile free layout: (half mtp, slab s, col c) so a plane can
            # arrive as two 256-col DMAs (S1 starts on the first half) or as
            # one DMA (halves HWDGE / issue load for later planes).
            def load_plane(t, sl, split):
                if split:
                    for mtp in range(2):
                        nc.sync.dma_start(
                            t[:, 1024 * mtp : 1024 * mtp + 1024]
                            .rearrange("p (s c) -> p s c", s=4),
                            img_d[sl, :, 256 * mtp : 256 * mtp + 256]
                            .rearrange("(s p) c -> p s c", p=128),
                        )
                else:
                    nc.sync.dma_start(
                        t[:].rearrange("p (m s c) -> p m s c", m=2, s=4),
                        img_d[sl].rearrange("(s p) (m c) -> p m s c", p=128, m=2),
                    )

            def xslice(t, w, mt):
                """[128,128] stationary slice: slab w, cols 128mt..128mt+128."""
                base = 1024 * (mt // 2) + 256 * w + 128 * (mt % 2)
                return t[:, base : base + 128]

            wpk = wp.tile([128, NW], F16, tag="wpk")
            # s13w ships first (S1 needs s1w within ~1.5us); s2w|s4w follow
            # as one merged DMA after the image loads are queued
            nc.sync.dma_start(wpk[:, 0:256], wpack_d[:, 0:256])
            s1w = wpk[:, 0:128]
            s3w = wpk[:, 128:256]
            s2w = wpk[:, 256 : 256 + 9 * 128]
            s4w = wpk[:, 256 + 9 * 128 : 256 + 9 * 128 + 7 * 128]
            s2yr = wpk[:, 256 + 16 * 128 : 256 + 20 * 128]

            # PE warmup: junk matmuls on a memset tile (no DMA dependency)
            # keep the tensor engine continuously busy through the preamble
            # and input-load head, so the p-state ramp completes before the
            # real pipeline starts.
            wgarb = wp.tile([128, 256], F16, tag="wgarb")
            nc.vector.memset(wgarb[:], 0.0)

            def s1fill(n):
                """Filler matmuls: absorb input-DMA pacing gaps and hold the
                PE ramp. Fresh pool tiles so psA rotation is not pinned."""
                done = 0
                while done < n:
                    k = min(4, n - done)
                    wu = psAp.tile([128, 1024], F32, tag="psA")
                    for r in range(k):
                        nc.tensor.matmul(
                            wu[:, 256 * r : 256 * r + 256],
                            wgarb[:, 0:128], wgarb[:],
                            start=True, stop=True,
                        )
                    done += k

            s1fill(QCONF.get("warm", 12))

            early_imgs = []
            for _c in range(3):
                _t = imp.tile([128, 2048], F16, tag=f"x{_c}")
                early_imgs.append(_t)
            if QCONF.get("hmajor"):
                for _mtp in range(2):
                    for _c in range(3):
                        nc.sync.dma_start(
                            early_imgs[_c][:, 1024 * _mtp : 1024 * _mtp + 1024]
                            .rearrange("p (s c) -> p s c", s=4),
                            img_d[_c, :, 256 * _mtp : 256 * _mtp + 256]
                            .rearrange("(s p) c -> p s c", p=128),
                        )
            else:
                for _c in range(3):
                    load_plane(early_imgs[_c], _c, split=True)
            qpk = wp.tile([128, NQ], F32, tag="qpk")
            nc.sync.dma_start(qpk[:], qp_d)
            # split upload: channel-0's three mix blocks land first so S2 can
            # start as soon as image 0 is resident
            nc.sync.dma_start(wpk[:, 256 : 256 + 3 * 128],
                              wpack_d[:, 256 : 256 + 3 * 128])
            nc.sync.dma_start(wpk[:, 256 + 3 * 128 :],
                              wpack_d[:, 256 + 3 * 128 :])
            O_RQ, O_QQ, O_DCA, O_S3B = 0, SLICES * 8, 16 * SLICES, 17 * SLICES
            # fp16 copy of the dequant patterns for the all-fp16 dequant mult
            qq16 = wp.tile([128, SLICES * 8], F16, tag="qq16")
            nc.scalar.activation(qq16[:], qpk[:, O_QQ : O_QQ + SLICES * 8], COPY)

            state = {}

            def _eng(code):
                return {"A": nc.scalar, "P": nc.gpsimd, "V": nc.vector}[code]

            def _copy(dst, src, code, bias=None):
                """psum->sbuf copy on A/V; 2-char code splits halves across
                two engines so the copy latency (psum rotation) halves."""
                n = dst.shape[1]
                parts = ([(0, n)] if len(code) == 1 else
                         [(0, n // 2), (n // 2, n)])
                for (lo, hi), eng in zip(parts, code):
                    d, s = dst[:, lo:hi], src[:, lo:hi]
                    if eng == "A":
                        if bias is not None:
                            nc.scalar.activation(d, s, IDENT, bias=bias)
                        else:
                            nc.scalar.activation(d, s, COPY)
                    else:
                        if bias is not None:
                            nc.vector.tensor_scalar_add(d, s, bias)
                        else:
                            nc.vector.tensor_scalar_add(d, s, 0.0)

            def s_load(im):
                if im == 0:
                    state[("x", 0)] = early_imgs
                    return
                xt = []
                for c in range(3):
                    t = imp.tile([128, 2048], F16, tag=f"x{c}")
                    load_plane(t, 3 * im + c, split=bool(QCONF.get("isplit", 0)))
                    xt.append(t)
                state[("x", im)] = xt

            def s_load_hmajor(im):
                """Issue all half-0 DMAs before half-1: the sp0 chain can
                start after ~3 half-plane transfers instead of 5."""
                xt = [imp.tile([128, 2048], F16, tag=f"x{c}") for c in range(3)]
                for mtp in range(2):
                    for c in range(3):
                        nc.sync.dma_start(
                            xt[c][:, 1024 * mtp : 1024 * mtp + 1024]
                            .rearrange("p (s c) -> p s c", s=4),
                            img_d[3 * im + c, :, 256 * mtp : 256 * mtp + 256]
                            .rearrange("(s p) c -> p s c", p=128),
                        )
                state[("x", im)] = xt

            def s1(im, chans=(0, 1, 2), mtps=(0, 1)):
                """Vertical DCT: per (c, mt-pair) one [128,1024] psum of 8
                fp16 matmuls, then one psum->sbuf fp16 copy."""
                xt = state[("x", im)]
                g1 = state.setdefault(("g1", im), [None, None, None])
                for c in chans:
                    if mtps[0] == 0:
                        g_t = g1p.tile([128, 2048], F16, tag=f"g1_{c}")
                        g1[c] = g_t
                    for mtp in mtps:
                        ps = psAp.tile([128, 1024], F32, tag="psA")
                        for k in range(2):
                            mt = 2 * mtp + k
                            for w in range(4):
                                nc.tensor.matmul(
                                    ps[:, 512 * k + 128 * w : 512 * k + 128 * w + 128],
                                    xslice(xt[c], w, mt),
                                    s1w,
                                    start=True, stop=True,
                                )
                        dst = g1[c][:, 1024 * mtp : 1024 * mtp + 1024]
                        _copy(dst, ps[:], _pick(QCONF["s1cp"], 6 * im + 2 * c + mtp))
                    s1fill(QCONF.get("s1fill", 0))

            def s2q(im, chans=(0, 1, 2), sps=(0, 1)):
                """Horizontal DCT + color mix + quantization.

                Per (c, sp): [128,1024] psum from 2x3 fp16 matmuls, then
                TT1 (psum * 1/q -> fp16).  Magic round (two 4x fp16 TSPs) and
                the fp16 dequant TT run per QCONF qgran granularity."""
                g1 = state[("g1", im)]
                dqt = state.setdefault(("q", im), [None, None, None])
                for c in chans:
                    if sps[0] == 0:
                        tq = tqp.tile([128, 2048], F16, tag=f"tq_{c}")
                        dq = dqp.tile([128, 2048], F16, tag=f"dq_{c}")
                        state[("tq", im, c)] = tq
                        dqt[c] = dq
                    tq = state[("tq", im, c)]
                    dq = dqt[c]
                    sl = 3 * im + c
                    rqv = qpk[:, O_RQ + 8 * sl : O_RQ + 8 * sl + 8].rearrange("p (o j) -> p o j", o=1).broadcast_to((128, 256, 8))
                    qqv16 = qq16[:, 8 * sl : 8 * sl + 8].rearrange("p (o j) -> p o j", o=1).broadcast_to((128, 256, 8))
                    yre = QCONF.get("yreuse") and c > 0
                    if QCONF.get("yreuse") and c == 0 and sps[0] == 0:
                        fy_t = fyp.tile([128, 2048], F16, tag="fy")
                        state[("fy", im)] = fy_t
                    for sp_ in sps:
                        ps = psBp.tile([128, 1024], F32, tag="psB")
                        for k in range(2):
                            s = 2 * sp_ + k
                            if yre:
                                # chroma = a*(data - Y): one plane stream +
                                # one F_Y passthrough (scaled identity)
                                src = 2 if c == 1 else 0
                                yo = 256 * (c - 1)
                                nc.tensor.matmul(
                                    ps[:, 512 * k : 512 * k + 512],
                                    s2yr[:, yo : yo + 128],
                                    g1[src][:, 512 * s : 512 * s + 512],
                                    start=True, stop=False,
                                )
                                nc.tensor.matmul(
                                    ps[:, 512 * k : 512 * k + 512],
                                    s2yr[:, yo + 128 : yo + 256],
                                    state[("fy", im)][:, 512 * s : 512 * s + 512],
                                    start=False, stop=True,
                                )
                                continue
                            for cp in range(3):
                                nc.tensor.matmul(
                                    ps[:, 512 * k : 512 * k + 512],
                                    s2w[:, 128 * (3 * c + cp) : 128 * (3 * c + cp) + 128],
                                    g1[cp][:, 512 * s : 512 * s + 512],
                                    start=(cp == 0), stop=(cp == 2),
                                )
                        ts = tq[:, 1024 * sp_ : 1024 * sp_ + 1024]
                        if QCONF.get("yreuse") and c == 0:
                            _copy(state[("fy", im)][:, 1024 * sp_ : 1024 * sp_ + 1024],
                                  ps[:], _pick(QCONF.get("fycp", ["A"]), 2 * im + sp_))
                        # TT1: psum * (1/q) -> fp16 (the only psum-port op)
                        nc.vector.tensor_tensor(
                            ts.rearrange("p (a j) -> p a j", j=8),
                            ps[:].rearrange("p (a j) -> p a j", j=8),
                            rqv[:, 0:128, :], op=mybir.AluOpType.mult,
                        )
                        if c == 0:
                            deng = _pick(QCONF["dca"], 2 * im + sp_)
                            dcacol = qpk[:, O_DCA + sl : O_DCA + sl + 1]
                            if deng == "A":
                                nc.scalar.activation(
                                    ts[:, 0:1024:8], ts[:, 0:1024:8], IDENT,
                                    bias=dcacol)
                            else:
                                _eng(deng).tensor_scalar_add(
                                    ts[:, 0:1024:8], ts[:, 0:1024:8], dcacol)
                    tc_ = 3 * im + c

                    def _magic_deq(xs, ds, qv, ti):
                        # fp16 magic round: the fp16 stores do the rounding
                        m1 = _pick(QCONF["m1"], ti)
                        if m1 == "A":
                            nc.scalar.activation(xs, xs, COPY, bias=MAGIC16)
                        else:
                            _eng(m1).tensor_scalar_add(xs, xs, MAGIC16)
                        m2 = _pick(QCONF["m2"], ti)
                        if m2 == "A":
                            nc.scalar.activation(xs, xs, COPY, bias=-MAGIC16)
                        else:
                            _eng(m2).tensor_scalar_sub(xs, xs, MAGIC16)
                        # dequant: all-fp16 2x tensor_tensor by the q pattern
                        _eng(_pick(QCONF["deq"], ti)).tensor_tensor(
                            ds.rearrange("p (a j) -> p a j", j=8),
                            xs.rearrange("p (a j) -> p a j", j=8),
                            qv, op=mybir.AluOpType.mult,
                        )

                    if QCONF.get("qgran", "ch") == "sp" or sps != (0, 1):
                        for sp_ in sps:
                            _magic_deq(tq[:, 1024 * sp_ : 1024 * sp_ + 1024],
                                       dq[:, 1024 * sp_ : 1024 * sp_ + 1024],
                                       qqv16[:, 0:128, :], 2 * tc_ + sp_)
                    else:
                        _magic_deq(tq[:], dq[:], qqv16, tc_)

            def s3(im, mtps=(0, 1), chans=(0, 1, 2)):
                """Horizontal inverse DCT (fp16 form-b). Needs all of dq[c]."""
                dqt = state[("q", im)]
                g3 = state.setdefault(("g3", im), [None, None, None])
                for mtp in mtps:
                    for c in (QCONF.get("s3ord", chans) if len(chans) == 3 else chans):
                        if mtp == 0:
                            g3_t = g3p.tile([128, 2048], F16, tag=f"g3_{c}")
                            g3[c] = g3_t
                        ps = psAp.tile([128, 1024], F32, tag="psA")
                        for k in range(2):
                            mt = 2 * mtp + k
                            for c2 in range(4):
                                nc.tensor.matmul(
                                    ps[:, 512 * k + 128 * c2 : 512 * k + 128 * c2 + 128],
                                    dqt[c][:, 512 * c2 + 128 * mt : 512 * c2 + 128 * mt + 128],
                                    s3w,
                                    start=True, stop=True,
                                )
                        g3s = g3[c][:, 1024 * mtp : 1024 * mtp + 1024]
                        _copy(g3s, ps[:], _pick(QCONF["s3cp"], 6 * im + 2 * c + mtp),
                              bias=(qpk[:, O_S3B : O_S3B + 1] if c == 0 else None))

            def s4(im, sps=(0, 1), chans=None):
                """Vertical inverse DCT + inverse color mix + store.

                Slab-pair sp only needs g3[:, 1024sp:+1024] = s3(im, mtp=sp).
                One plain psum->sbuf fp16 copy (clip runs on host), one
                [128,1024] DMA per channel."""
                g3 = state[("g3", im)]
                for sp_ in sps:
                    if ("ot", im, sp_) not in state:
                        ot_new = op.tile([128, 3072], F16, tag="ot")
                        state[("ot", im, sp_)] = ot_new
                    ot_t = state[("ot", im, sp_)]
                    for chan in (chans if chans is not None
                                 else QCONF.get("s4ord", (0, 1, 2))):
                        terms = _S4TERMS[chan]
                        ps = psBp.tile([128, 1024], F32, tag="psB")
                        for k in range(2):
                            s = 2 * sp_ + k
                            for ti, (csrc, _) in enumerate(terms):
                                wi = s4idx[(chan, csrc)]
                                nc.tensor.matmul(
                                    ps[:, 512 * k : 512 * k + 512],
                                    s4w[:, 128 * wi : 128 * wi + 128],
                                    g3[csrc][:, 512 * s : 512 * s + 512],
                                    start=(ti == 0), stop=(ti == len(terms) - 1),
                                )
                        ots = ot_t[:, 1024 * chan : 1024 * chan + 1024]
                        ci = 6 * im + 3 * sp_ + chan
                        _copy(ots, ps[:], _pick(QCONF["s4cp"], ci))
                        qeng = {"S": nc.sync, "A": nc.scalar, "P": nc.gpsimd,
                                "V": nc.vector}
                        ti_ = 2 * im + sp_
                        ndone = state.get(("otn", im, sp_), 0) + 1
                        state[("otn", im, sp_)] = ndone
                        tf = QCONF.get("tailfine")
                        if tf and (im, sp_) == (1, 1):
                            # final tile: halve copies across A+V and ship
                            # each half on its own queue so the drain after
                            # the last matmul is as short as possible
                            od = out_d[2 * im + sp_, :, 1024 * chan : 1024 * chan + 1024]
                            _copy(ots, ps[:], tf[0])
                            q0, q1 = qeng[tf[1]], qeng[tf[2]]
                            q0.dma_start(od[:, 0:512], ots[:, 0:512])
                            q1.dma_start(od[:, 512:1024], ots[:, 512:1024])
                            continue
                        if _pick(QCONF.get("obatch", ["ch"]), ti_) == "ch":
                            oeng = qeng[_pick(QCONF.get("oq", ["S"]), ci)]
                            oeng.dma_start(
                                out_d[2 * im + sp_, :, 1024 * chan : 1024 * chan + 1024],
                                ots,
                            )
                        elif ndone == 3:
                            # batched: one [128,3072] DMA after the last copy
                            oeng = qeng[_pick(QCONF.get("oq", ["S"]), ti_)]
                            oeng.dma_start(out_d[2 * im + sp_], ot_t[:])

            # software-pipelined schedule; S3/S4 interleave per column-pair
            # so outputs stream early instead of flushing at the tail.
            s_load(0)
            s_load(1)
            if QCONF["sched"] == "fwd2":
                # both images' forward stages first (wide window for the
                # quant chains to hide behind PE work), then the inverses
                s1(0)
                s2q(0)
                s1(1)
                s2q(1)
                for im in (0, 1):
                    s3(im, mtps=(0,))
                    s4(im, sps=(0,))
                    s3(im, mtps=(1,))
                    s4(im, sps=(1,))
            elif QCONF["sched"] == "fwd2i":
                # S2(0) tiles interleaved with S1(1) channels: PE keeps S2
                # work in flight while ACT copies S1(1) psums
                s1(0)
                s2q(0, chans=(0,))
                s1(1, chans=(0,))
                s2q(0, chans=(1,))
                s1(1, chans=(1,))
                s2q(0, chans=(2,))
                s1(1, chans=(2,))
                s2q(1)
                for im in (0, 1):
                    s3(im, mtps=(0,))
                    s4(im, sps=(0,))
                    s3(im, mtps=(1,))
                    s4(im, sps=(1,))
            elif QCONF["sched"] == "half":
                # sp-half pipelining: the sp0 chain starts after only the
                # three half-0 plane transfers (use with hmajor=1)
                s1(0, mtps=(0,))
                s2q(0, sps=(0,))
                s1(0, mtps=(1,))
                s2q(0, sps=(1,))
                s1(1, mtps=(0,))
                s2q(1, sps=(0,))
                s1(1, mtps=(1,))
                s2q(1, sps=(1,))
                for im in (0, 1):
                    s3(im, mtps=(0,))
                    s4(im, sps=(0,))
                    s3(im, mtps=(1,))
                    s4(im, sps=(1,))
            elif QCONF["sched"] == "halfi":
                # like half, with im1's S1 woven into im0's S2 tiles
                s1(0, mtps=(0,))
                s2q(0, sps=(0,))
                s1(0, mtps=(1,))
                s2q(0, sps=(1,), chans=(0,))
                s1(1, mtps=(0,), chans=(0,))
                s2q(0, sps=(1,), chans=(1,))
                s1(1, mtps=(0,), chans=(1,))
                s2q(0, sps=(1,), chans=(2,))
                s1(1, mtps=(0,), chans=(2,))
                s2q(1, sps=(0,))
                s1(1, mtps=(1,))
                s2q(1, sps=(1,))
                for im in (0, 1):
                    s3(im, mtps=(0,))
                    s4(im, sps=(0,))
                    s3(im, mtps=(1,))
                    s4(im, sps=(1,))
            elif QCONF["sched"] == "fine":
                # inverse at channel granularity: S4(R) starts once g3 Y+Cr
                # exist, S4(B) after Cb, G last — shorter s3->s4 latency
                s1(0)
                s2q(0)
                s1(1)
                s2q(1)
                for im in (0, 1):
                    for h in (0, 1):
                        s3(im, mtps=(h,), chans=(0,))
                        s3(im, mtps=(h,), chans=(2,))
                        s4(im, sps=(h,), chans=(0,))
                        s3(im, mtps=(h,), chans=(1,))
                        s4(im, sps=(h,), chans=(2,))
                        s4(im, sps=(h,), chans=(1,))
            elif QCONF["sched"] == "fine2":
                # like fine but forward interleaved as in fwd2i
                s1(0)
                s2q(0, chans=(0,))
                s1(1, chans=(0,))
                s2q(0, chans=(1,))
                s1(1, chans=(1,))
                s2q(0, chans=(2,))
                s1(1, chans=(2,))
                s2q(1)
                for im in (0, 1):
                    for h in (0, 1):
                        s3(im, mtps=(h,), chans=(0,))
                        s3(im, mtps=(h,), chans=(2,))
                        s4(im, sps=(h,), chans=(0,))
                        s3(im, mtps=(h,), chans=(1,))
                        s4(im, sps=(h,), chans=(2,))
                        s4(im, sps=(h,), chans=(1,))
            elif QCONF["sched"] == "zip":
                # im1's S1 fills PE while im0's quant drains; one im0 inverse
                # chunk is held back to cover im1's quant drain.
                s1(0)
                s2q(0)
                s3(0, mtps=(0,))
                s1(1, chans=(0,))
                s4(0, sps=(0,))
                s1(1, chans=(1,))
                s3(0, mtps=(1,))
                s1(1, chans=(2,))
                s2q(1)
                s4(0, sps=(1,))
                s3(1, mtps=(0,))
                s4(1, sps=(0,))
                s3(1, mtps=(1,))
                s4(1, sps=(1,))
            elif QCONF["sched"] == "zip2":
                # finer interleave: S1(1) channels woven into S2(0) tiles
                s1(0)
                s2q(0, chans=(0,))
                s1(1, chans=(0,))
                s2q(0, chans=(1,))
                s1(1, chans=(1,))
                s2q(0, chans=(2,))
                s1(1, chans=(2,))
                s3(0, mtps=(0,))
                s4(0, sps=(0,))
                s2q(1, chans=(0,))
                s3(0, mtps=(1,))
                s2q(1, chans=(1,))
                s4(0, sps=(1,))
                s2q(1, chans=(2,))
                s3(1, mtps=(0,))
                s4(1, sps=(0,))
                s3(1, mtps=(1,))
                s4(1, sps=(1,))
            elif QCONF["sched"] == "hyb":
                # image-1 forward partially interleaved so image-0 outputs
                # start early and output DMA spreads across the whole run
                s1(0)
                s2q(0)
                s1(1)
                s2q(1, chans=(0,))
                s3(0, mtps=(0,))
                s4(0, sps=(0,))
                s2q(1, chans=(1,))
                s3(0, mtps=(1,))
                s4(0, sps=(1,))
                s2q(1, chans=(2,))
                s3(1, mtps=(0,))
                s4(1, sps=(0,))
                s3(1, mtps=(1,))
                s4(1, sps=(1,))
            else:
                s1(0)
                s2q(0)
                s1(1)
                s3(0, mtps=(0,))
                s4(0, sps=(0,))
                s2q(1, chans=(0, 1))
                s3(0, mtps=(1,))
                s4(0, sps=(1,))
                s2q(1, chans=(2,))
                s3(1, mtps=(0,))
                s4(1, sps=(0,))
                s3(1, mtps=(1,))
                s4(1, sps=(1,))
    nc.compile()
    return nc, hc


_COMPILED = None


def _get_compiled():
    global _COMPILED
    if _COMPILED is None:
        _COMPILED = _trace()
    return _COMPILED


def kernel(img, quality):
    img = np.asarray(img)
    quality = int(np.asarray(quality))
    nc, hc = _get_compiled()

    img16 = np.ascontiguousarray(img.astype(np.float16))
    in_maps = []
    for core in range(N_CORES):
        qpack = _quant_inputs(quality, core, hc["s3b"])
        shard = np.ascontiguousarray(
            img16[IMGS_PER_CORE * core : IMGS_PER_CORE * (core + 1)].reshape(SLICES, 512, 512)
        )
        in_maps.append({
            "img": shard, "qpack": qpack, "wpack": hc["wpack"],
        })

    res = run_bass_kernel_spmd(nc, in_maps, core_ids=list(range(N_CORES)))
    # wire layout per core: [im*2+sp, p, (chan, slab, col)] -> [2,3,512,512]
    out = np.stack([res.results[c]["out"] for c in range(N_CORES)])
    out = out.reshape(N_CORES, IMGS_PER_CORE, 2, 128, 3, 2, 512)
    out = out.transpose(0, 1, 4, 2, 5, 3, 6)   # [core, im, ch, sp, s, p, col]
    out = np.ascontiguousarray(out).reshape(BS, 3, 512, 512).astype(np.float32)
    return np.clip(out, 0.0, 1.0)


if __name__ == "__main__":
    rng = np.random.default_rng(0)
    x = rng.random((BS, 3, 512, 512), dtype=np.float32)
    y = kernel(x, 80)
    print("kernel ran:", y.shape, y.dtype, float(y.min()), float(y.max()))
